# revision 18
# baseline (speedup 1.0000x reference)
"""Trainium2 Bass kernel for single-head causal attention.

Problem: x[4, 4096, 100], Wq/Wk/Wv[100, 64] ->
         softmax(tril(x@Wq @ (x@Wk)^T / 8)) @ (x@Wv)   -> [4, 4096, 64]

Sharding (8 cores, SPMD single program):
  core = 2*b + c: batch b in 0..3, key-parity c in 0..1.
  Each core handles ALL 4096 queries of its batch but only the keys/values at
  global rows {c, c+2, c+4, ...} (2048 of them). This keeps the causal
  structure IDENTICAL across cores (local key tile kk is attended by global
  query columns q >= 256*kk, for both parities), so one program serves all 8
  cores; the one-column parity offset lives in a tiny [128, 256] data mask.
  Softmax is computed without max-subtraction (scores are bounded ~|s|<=9
  after the 1/8 scale; exp also gets a -4*ln2 bias so E=exp(s)/16 fits fp8
  e4m3 range, max ~340 < 448), so the two half-key partials combine on the
  host as (num_A + num_B) / (den_A + den_B); the 1/16 scales out in the ratio.

Per-core program (flash-attention style, scores kept transposed):
  qT = Wq^T x^T  [64, 4096],  kT = Wk^T xkv^T [64, 2048]  (bf16, NOT
  column-duplicated: traced paired S^T matmuls never actually overlap in the
  PE halves, so the duplication bought nothing)
  V[kk] = x_kv@Wv per 128-key tile, stored THREE ways:
    v1bf  [128, 65] bf16  ([V | 1] ones col) -- used on the 2 boundary
          (diagonal) tiles of each strip, where masking happens
    v8hi/v8lo [128, 64] fp8e4m3 hi + residual -- used off the diagonal
  for each 512-query block qm (8 strips), key tiles 0..2qm+1 in PAIRS
  (one exp per pair; the last pair is the diagonal):
    S^T[tile] = kT[tile]^T-block @ qT-block   (bf16 in, PSUM f32 [128,512])
    off-diag pairs: E = exp(S^T/8 - 4ln2) -> fp8 ring [128, 24, 512]
        (strips alternate ring halves: pair AVs are emitted two groups late,
        which can cross into the next strip, and must not read re-written
        slots)
    diag pair: E -> bf16 (packed: last tile only 256 q-cols), then masked
        (DVE) with the parity mask
    AV (two groups late so the in-order PE streams ahead of exp):
      off-diag pair: three fp8 DoubleRow matmuls -- V8hi-pair + V8lo-pair
        accumulate num in o_t[0:64]; ones-pair accumulates den in a
        DEDICATED den bank [0:64] (the dual-fp8 matmul ISA requires dst
        partition base 0 -- probed: bases 32/64 fail s3d3_mm_valid_dst_
        partition). DoubleRow contracts 2x128 keys at 0.5 cycles/row: 768
        PE cycles/pair vs 1024 bf16, at bf16-identical accuracy (fp8 E
        quantization noise cancels in the num/den softmax ratio; V8hi+V8lo
        recovers full V precision).
      diag pair: bf16 matmuls with v1bf ([V|1], packed final tile) into
        o_t[0:65] (row 64 pre-zeroed by gpsimd memset; its den adds to the
        DR den at flush).
    flush: ob[0:64] <- o_t num (DVE; scalar on the last strip),
        ob[64] <- den bank row 0 (gpsimd; copying it early also frees the
        den bank for the next strip) then += o_t[64] (gpsimd), DMA on the
        sync queue.

Scheduling notes (HW-traced):
  - PE HAM clock gate: dependency-free iota-fed warmup matmuls bridge the
    DMA-latency head so the PE reaches 2.4 GHz before real work.
  - Head DMAs: first-use tensors split across the sync (w3, xq0) and scalar
    (xkv0) queues; mid-kernel chunks ride gpsimd (SWDGE jitters ~2us) and
    sync.
  - PSUM budget (8 banks): psA 2x[128,2,512] S^T pairs (4) + 2x[128,512]
    out (2) + den bank (1) + projection bank (1).
  - Projections dribble behind the exps: qT/kT in [64,512] halves of the
    projection bank; V tiles in the diagonal groups' free packed region
    (slot1 cols 256:512). Static schedule lands every projection >=1 group
    ahead of first use (asserted at build).
  - Tail: last strip's num copy on the scalar queue (idle then), out-DMA on
    the sync queue (shorter DGE issue than scalar).
"""

import os
from collections import deque
from contextlib import ExitStack

import numpy as np

B, T, E, H = 4, 4096, 100, 64
TK = T // 2  # keys per core
NKT = TK // 128  # 16 local key tiles
NQB = T // 512  # 8 query blocks
N_CORES = 8
N_WARM = 12  # PE warmup matmuls (HAM clock-gate bridge over the DMA head)

# qT/kT projection pairs: one pair drained per group (strip >= 1), into the
# [0:64]/[64:128] partition-halves of the projection bank. Need-by: k1 by
# strip2-g2, k2 by strip4-g4, k3 by strip6-g6, qTj by strip j.
QK_PAIRS = [
    [("q", 2), ("q", 3)], [("k", 1), ("q", 4)], [("q", 5), ("k", 2)],
    [("q", 6), ("q", 7)], [("k", 3)],
]
# V-projection pairs: drained at each strip's diagonal group into its free
# packed region. Pair (2m+2, 2m+3) lands at strip m's diag, first consumed
# by strip m+1's diag (v1bf) / strip m+2 (v8).
V_PAIRS = [(2, 3), (4, 5), (6, 7), (8, 9), (10, 11), (12, 13), (14, 15)]

_CACHE = {}


def _mask_np(c):
    """mask[i, j] = 1 if global key (2i+c) <= query col offset j else 0."""
    import ml_dtypes

    i = np.arange(128)[:, None]
    j = np.arange(256)[None, :]
    return (j >= 2 * i + c).astype(ml_dtypes.bfloat16)


def _build():
    if "nc" in _CACHE:
        return _CACHE["nc"]

    import concourse.bacc as bacc
    import concourse.tile as tile
    from concourse import mybir
    from concourse.bass import ts, ds

    f32 = mybir.dt.float32
    bf16 = mybir.dt.bfloat16
    fp8 = mybir.dt.float8e4
    Exp = mybir.ActivationFunctionType.Exp
    Mult = mybir.AluOpType.mult
    Add = mybir.AluOpType.add
    Sub = mybir.AluOpType.subtract
    DR = mybir.MatmulPerfMode.DoubleRow
    EXP_BIAS = float(-4.0 * np.log(2.0))

    nc = bacc.Bacc("TRN2", target_bir_lowering=False, debug=False,
                   num_devices=N_CORES)

    xq_d = nc.dram_tensor("xq", [E, T], bf16, kind="ExternalInput").ap()
    xkv_d = nc.dram_tensor("xkv", [E, TK], bf16, kind="ExternalInput").ap()
    w3_d = nc.dram_tensor("w3", [E, 3 * H], bf16, kind="ExternalInput").ap()
    mask_d = nc.dram_tensor("mask", [128, 256], bf16,
                            kind="ExternalInput").ap()
    out_d = nc.dram_tensor("out", [H + 1, T], f32, kind="ExternalOutput").ap()

    with tile.TileContext(nc) as tc, ExitStack() as ctx:
        sb = ctx.enter_context(tc.tile_pool(name="sb", bufs=1))
        ep = ctx.enter_context(tc.tile_pool(name="ep", bufs=2))
        ob_p = ctx.enter_context(tc.tile_pool(name="ob", bufs=2))
        psA = ctx.enter_context(tc.tile_pool(name="psA", bufs=2, space="PSUM"))
        ps_o = ctx.enter_context(tc.tile_pool(name="ps_o", bufs=2,
                                              space="PSUM"))
        ps_dn = ctx.enter_context(tc.tile_pool(name="ps_dn", bufs=1,
                                               space="PSUM"))
        ps_pr = ctx.enter_context(tc.tile_pool(name="ps_pr", bufs=1,
                                               space="PSUM"))

        xq_t = sb.tile([E, T], bf16)
        xkv_t = sb.tile([E, TK], bf16)
        w3_t = sb.tile([E, 3 * H], bf16)
        mask_t = sb.tile([128, 256], bf16)
        wq_t = w3_t[:, 0:H]
        wk_t = w3_t[:, H:2 * H]
        wv_t = w3_t[:, 2 * H:3 * H]
        qT_t = sb.tile([64, T], bf16)
        kT_t = sb.tile([64, TK], bf16)
        v1bf_t = sb.tile([128, NKT, H + 1], bf16)
        v8hi_t = sb.tile([128, NKT, H], fp8)
        v8lo_t = sb.tile([128, NKT, H], fp8)
        ones8_t = sb.tile([128, 2, H], fp8)
        ering_t = sb.tile([128, 24, 512], fp8)
        warm_t = sb.tile([128, 8], f32)
        wmm_t = sb.tile([128, 512], bf16)
        bias_t = sb.tile([128, 1], f32)  # exp bias: -4*ln2 (E = exp(s)/16)

        den_t = ps_dn.tile([128, 512], f32)
        proj_t = ps_pr.tile([128, 512], f32)

        # ---- PE warmup (HAM clock gate) + input DMAs -------------------
        nc.gpsimd.iota(wmm_t, [[1, 512]], channel_multiplier=1,
                       allow_small_or_imprecise_dtypes=True)
        head_t = psA.tile([128, 2, 512], f32, tag="s")
        for _ in range(N_WARM):
            nc.tensor.matmul(head_t[:, 1], wmm_t[:, 0:128], wmm_t,
                             start=True, stop=True)

        nc.sync.dma_start(out=w3_t, in_=w3_d)
        nc.scalar.dma_start(out=xkv_t[:, 0:512], in_=xkv_d[:, 0:512])
        nc.sync.dma_start(out=xq_t[:, 0:512], in_=xq_d[:, 0:512])
        nc.gpsimd.dma_start(out=mask_t, in_=mask_d)
        nc.sync.dma_start(out=xq_t[:, 512:1024], in_=xq_d[:, 512:1024])
        nc.gpsimd.dma_start(out=xkv_t[:, 512:1024], in_=xkv_d[:, 512:1024])
        nc.sync.dma_start(out=xq_t[:, 1024:2048], in_=xq_d[:, 1024:2048])
        nc.gpsimd.dma_start(out=xkv_t[:, 1024:2048], in_=xkv_d[:, 1024:2048])
        nc.sync.dma_start(out=xq_t[:, 2048:3072], in_=xq_d[:, 2048:3072])
        nc.gpsimd.dma_start(out=xq_t[:, 3072:4096], in_=xq_d[:, 3072:4096])

        # First ACT instruction early: overlaps the ~2.7us exp-table load
        # with input DMA.
        nc.vector.memset(warm_t, 0.0)
        nc.vector.memset(bias_t, EXP_BIAS)
        nc.scalar.activation(out=warm_t, in_=warm_t, func=Exp)
        nc.vector.memset(v1bf_t[:, :, H], 1.0)
        nc.vector.memset(ones8_t, 1.0)

        # ---- projections ----------------------------------------------
        qT_done = [False] * NQB
        kT_done = [False] * (TK // 512)
        v_done = [False] * NKT

        def proj_q(j, ps64):
            qT_done[j] = True
            nc.tensor.matmul(ps64, wq_t, xq_t[:, ts(j, 512)],
                             start=True, stop=True)
            nc.vector.tensor_copy(qT_t[:, ts(j, 512)], ps64)

        def proj_k(j, ps64):
            kT_done[j] = True
            nc.tensor.matmul(ps64, wk_t, xkv_t[:, ts(j, 512)],
                             start=True, stop=True)
            nc.vector.tensor_copy(kT_t[:, ts(j, 512)], ps64)

        def proj_v(kk, psv):
            v_done[kk] = True
            nc.tensor.matmul(psv, xkv_t[:, ts(kk, 128)], wv_t,
                             start=True, stop=True)
            nc.vector.tensor_copy(v1bf_t[:, kk, :H], psv)
            if kk < NKT - 2:  # pairs 0..6 are consumed off-diagonal
                nc.vector.tensor_copy(v8hi_t[:, kk], psv)
                nc.vector.tensor_tensor(v8lo_t[:, kk], psv, v8hi_t[:, kk],
                                        Sub)

        # Head projections: kT0 FIRST (xkv0 lands earliest, scalar queue),
        # then qT0, in the projection bank halves; V0+V1 ride the warmup
        # tile's slot 1 (in-order PE serializes the WAW with the warmups).
        proj_k(0, proj_t[64:128])
        proj_q(0, proj_t[0:64])
        proj_v(0, head_t[:, 1, 0:64])
        proj_v(1, head_t[:, 1, 64:128])

        qk_pairs = deque(QK_PAIRS)
        v_pairs = deque(V_PAIRS)

        def fillers(n):
            # dependency-free PE work into the den bank's unused partition
            # half: the HAM clock gate re-throttles the PE to ~1.2 GHz
            # whenever it idles (v3 trace: every matmul ~1.5-2x slower once
            # the fp8 AVs opened gaps), so pad the queue to keep it busy.
            for _ in range(n):
                nc.tensor.matmul(den_t[64:128, 0:128], wmm_t[:, 0:64],
                                 wmm_t[:, 0:128], start=True, stop=True,
                                 skip_group_check=True)

        # ---- main attention loop --------------------------------------
        pend = deque()

        def emit_av(p):
            if p[0] == "dr":
                _, o_t, kk, rbase = p
                assert v_done[kk] and v_done[kk + 1], f"v proj sched {kk}"
                r = rbase + kk % 12
                rhs = ering_t[:, r:r + 2, :]
                nc.tensor.matmul(o_t[0:64], v8hi_t[:, kk:kk + 2], rhs,
                                 start=(kk == 0), stop=False, perf_mode=DR,
                                 skip_group_check=True)
                nc.tensor.matmul(o_t[0:64], v8lo_t[:, kk:kk + 2], rhs,
                                 start=False, stop=False, perf_mode=DR,
                                 skip_group_check=True)
                nc.tensor.matmul(den_t[0:64], ones8_t, rhs,
                                 start=(kk == 0), stop=False,
                                 perf_mode=DR, skip_group_check=True)
            else:
                _, o_t, eb, nkk, qm = p
                assert v_done[nkk - 2] and v_done[nkk - 1], f"v sched {nkk}"
                nc.tensor.matmul(o_t[:H], v1bf_t[:, nkk - 2, 0:H], eb[:, 0],
                                 start=(nkk == 2), stop=False,
                                 skip_group_check=True)
                nc.tensor.matmul(o_t[:H, 256:512], v1bf_t[:, nkk - 1, 0:H],
                                 eb[:, 1, 0:256], start=False, stop=True,
                                 skip_group_check=True)
                # diag den: ones-column bf16 matmuls accumulate onto the DR
                # den in the den bank row 0 (gpsimd cannot touch PSUM)
                nc.tensor.matmul(den_t[0:1], v1bf_t[:, 0, H:H + 1], eb[:, 0],
                                 start=(nkk == 2), stop=False,
                                 skip_group_check=True)
                nc.tensor.matmul(den_t[0:1, 256:512], v1bf_t[:, 0, H:H + 1],
                                 eb[:, 1, 0:256], start=False, stop=True,
                                 skip_group_check=True)
                ob = ob_p.tile([H + 1, 512], f32, tag="ob")
                if qm == NQB - 1:
                    nc.scalar.copy(ob[0:H], o_t[0:H])
                else:
                    nc.vector.tensor_copy(ob[0:H], o_t[0:H])
                # copying den early also frees the den bank for the next
                # strip's DR-den chain
                nc.vector.tensor_copy(ob[H:H + 1], den_t[0:1])
                nc.sync.dma_start(out=out_d[:, ds(512 * qm, 512)], in_=ob)

        for qm in range(NQB):
            nkk = 2 * qm + 2
            assert qT_done[qm], f"qT proj sched {qm}"
            o_t = ps_o.tile([128, 512], f32, tag="o")
            qs = qT_t[:, ds(512 * qm, 512)]
            for g in range(nkk // 2):
                tiles = [2 * g, 2 * g + 1]
                diag = tiles[-1] == nkk - 1
                s_t = psA.tile([128, 2, 512], f32, tag="s")
                for i, kk in enumerate(tiles):
                    assert kT_done[kk // 4], f"kT proj sched {kk}"
                    kts = kT_t[:, ts(kk, 128)]
                    if diag and kk == nkk - 1:
                        nc.tensor.matmul(s_t[:, i, 0:256], kts,
                                         qs[:, 256:512], start=True,
                                         stop=True)
                    else:
                        nc.tensor.matmul(s_t[:, i], kts, qs,
                                         start=True, stop=True)
                sf = s_t.rearrange("p a b -> p (a b)")
                if diag:
                    eb = ep.tile([128, 2, 512], bf16, tag="e")
                    ef = eb.rearrange("p a b -> p (a b)")
                    nc.scalar.activation(out=ef[:, :768], in_=sf[:, :768],
                                         func=Exp, scale=float(H) ** -0.5,
                                         bias=bias_t)
                else:
                    rbase = (qm % 2) * 12
                    r = rbase + tiles[0] % 12
                    ef = ering_t[:, r:r + 2, :].rearrange("p a b -> p (a b)")
                    nc.scalar.activation(out=ef, in_=sf,
                                         func=Exp, scale=float(H) ** -0.5,
                                         bias=bias_t)
                if qm == 0:
                    # qT1 spliced in behind strip 0's exp
                    proj_q(1, proj_t[0:64])
                elif qk_pairs:
                    for half, task in enumerate(qk_pairs.popleft()):
                        kind, j = task
                        ps64 = proj_t[64 * half:64 * (half + 1)]
                        (proj_q if kind == "q" else proj_k)(j, ps64)
                if diag:
                    if v_pairs:
                        # V pair into this group's free packed region
                        ka, kb = v_pairs.popleft()
                        proj_v(ka, s_t[:, 1, 256:320])
                        proj_v(kb, s_t[:, 1, 320:384])
                    # boundary masking against the causal diagonal
                    nc.vector.tensor_tensor(eb[:, 0, 0:256], eb[:, 0, 0:256],
                                            mask_t, Mult)
                    nc.vector.tensor_tensor(eb[:, 1, 0:256], eb[:, 1, 0:256],
                                            mask_t, Mult)
                    newp = ("bf", o_t, eb, nkk, qm)
                else:
                    newp = ("dr", o_t, tiles[0], (qm % 2) * 12)
                while len(pend) >= 2:
                    emit_av(pend.popleft())
                pend.append(newp)
                fillers(5)
        while pend:
            emit_av(pend.popleft())

    nc.compile()
    _CACHE["nc"] = nc
    return nc


def _bf16(a):
    import ml_dtypes

    return np.ascontiguousarray(a, dtype=np.float32).astype(ml_dtypes.bfloat16)


def _make_in_maps(x, Wq, Wk, Wv):
    import ml_dtypes

    x = np.asarray(x, dtype=np.float32)
    w3 = np.zeros((E, 3 * H), dtype=ml_dtypes.bfloat16)
    w3[:, 0:H] = _bf16(Wq)
    w3[:, H:2 * H] = _bf16(Wk)
    w3[:, 2 * H:3 * H] = _bf16(Wv)
    masks = [_mask_np(0), _mask_np(1)]
    in_maps = []
    for core in range(N_CORES):
        b, c = divmod(core, 2)
        in_maps.append({
            "xq": _bf16(x[b].T),
            "xkv": _bf16(x[b, c::2, :].T),
            "w3": w3,
            "mask": masks[c],
        })
    return in_maps


def _combine(results):
    out = np.empty((B, T, H), dtype=np.float32)
    for b in range(B):
        a = results[2 * b]["out"]
        bb = results[2 * b + 1]["out"]
        num = a[:H] + bb[:H]
        den = a[H] + bb[H]
        out[b] = (num / den).T
    return out


def run(x, Wq, Wk, Wv, trace=False):
    """Returns (output [4,4096,64] f32, exec_time_ns or None)."""
    from concourse.bass_utils import run_bass_kernel_spmd

    nc = _build()
    in_maps = _make_in_maps(x, Wq, Wk, Wv)
    res = run_bass_kernel_spmd(nc, in_maps, core_ids=list(range(N_CORES)),
                               trace=trace)
    return _combine(res.results), res


def kernel(x, Wq, Wk, Wv):
    out, _ = run(x, Wq, Wk, Wv, trace=False)
    return out
